# revision 1
# baseline (speedup 1.0000x reference)
"""Trainium2 Bass kernel for nn_AMK_Block (dense transformer block).

Sequence-parallel across 8 NeuronCores: each core owns 512 of the 4096
rows. QKV + RoPE + RMS-norm computed locally feature-major; K/V (with a
baked ones-column for the attention row-sum) are all-gathered; the
(elu+1)^2-kernel attention, W_o, SwiGLU+depthwise-conv FFN and final
norm run locally; per-core output shards are concatenated on the host.

All activations are kept feature-major (transposed): Y = X @ W becomes
Y^T = lhsT.T @ rhs with lhsT = W-slice, rhs = X^T, so weights load
straight from DRAM with no transposes.

elu(x)+1 == min(exp(x), 1 + relu(x)) (exp(x) >= 1+x everywhere), so the
attention kernel W = (elu(S)+1)^2 = min(exp(2S), (1+relu(S))^2) needs a
single transcendental pass; work is split across ACT/DVE/GPSIMD.

Collectives can't touch I/O tensors and SBUF<->internal-DRAM DMAs are
broken in this runtime, so collective data is staged through external
DRAM buffers with DRAM->DRAM hops on both sides.
"""
import sys
import numpy as np

sys.path.insert(0, "/opt/trn_rl_repo")

import ml_dtypes  # noqa: E402
import concourse.bass as bass  # noqa: E402
import concourse.mybir as mybir  # noqa: E402
from concourse import tile  # noqa: E402
from concourse.bass_utils import run_bass_kernel_spmd  # noqa: E402

BF16 = mybir.dt.bfloat16
F32 = mybir.dt.float32
AF = mybir.ActivationFunctionType
OP = mybir.AluOpType
bfdt = ml_dtypes.bfloat16

R = 8          # cores
N = 4096       # sequence
NL = N // R    # local rows = 512
D = 1024
H = 16
DH = 64
NK = D // 128  # 8 k-tiles of the d axis
INNER = 2816
CT = INNER // 128  # 22 channel tiles
KV_K = H * DH * NL          # K^T region elems (bf16)
KV_V = 4 * 128 * (H * 65)   # V_aug region elems
KV_TOT = KV_K + KV_V
HALO = CT * 2 * 128         # boundary staging elems per rank (bf16)


# ---------------------------------------------------------------- waitfix
def fix_sync_waits(nc, limit=1):
    """Walrus here allows at most 1 sem wait per instruction, and the
    runtime drops waits embedded in DMA instructions. Move excess waits
    onto same-engine NOPs inserted right before the instruction."""
    n_fixed = 0
    for f in nc.m.functions:
        for bb in f.blocks:
            insts = list(bb.instructions)
            out = []
            changed = False
            for inst in insts:
                si = inst.sync_info
                n_waits = len(si.on_wait) if (si is not None and si.on_wait) else 0
                is_dma = "DMA" in type(inst).__name__ or "DmaTranspose" in type(inst).__name__
                eff = 0 if is_dma else limit
                if n_waits > eff:
                    waits = list(si.on_wait)
                    keep = waits[-eff:] if eff > 0 else []
                    extra = waits[: len(waits) - eff]
                    for i in range(0, len(extra), limit):
                        ch = extra[i:i + limit]
                        nop_bi = nc.engines[inst.engine].nop(hint="waitsplit")
                        nop = nop_bi.ins if hasattr(nop_bi, "ins") else nop_bi
                        cur = nc.cur_bb.bb
                        cur_insts = list(cur.instructions)
                        assert cur_insts and cur_insts[-1].name == nop.name
                        cur.instructions = cur_insts[:-1]
                        nop.sync_info = mybir.SyncInfo(on_wait=list(ch), on_update=[])
                        out.append(nop)
                    inst.sync_info = mybir.SyncInfo(
                        on_wait=list(keep), on_update=list(si.on_update or [])
                    )
                    n_fixed += 1
                    changed = True
                out.append(inst)
            if changed:
                bb.instructions = out
    return n_fixed


# ---------------------------------------------------------------- build
def build_kernel():
    nc = bass.Bass()

    # register the non-default ACT scale/bias constants we use
    def reg_const(dtype, value):
        if (dtype, value) in nc.const_aps.aps:
            return
        t = nc.alloc_sbuf_tensor(f"const-{dtype.name}-{value}", [128, 1], dtype)
        nc.gpsimd.memset(t.ap(), value)
        nc.const_aps.aps[(dtype, value)] = t.ap()

    for v in (2.0, 1.0 / DH, 1.0 / D, 1e-5, DH * 1e-5, 1e-6):
        reg_const(F32, v)
    nc.all_engine_barrier()

    xt_e = nc.declare_dram_parameter("xt", [D, NL], F32, isOutput=False)
    wqkv_e = nc.declare_dram_parameter("wqkv", [D, 3 * D], BF16, isOutput=False)
    wo_e = nc.declare_dram_parameter("wo", [D, D], BF16, isOutput=False)
    wup_e = nc.declare_dram_parameter("wup", [D, 2 * INNER], BF16, isOutput=False)
    wdn_e = nc.declare_dram_parameter("wdn", [INNER, D], BF16, isOutput=False)
    cs_e = nc.declare_dram_parameter("cs", [DH, NL], BF16, isOutput=False)
    sn_e = nc.declare_dram_parameter("sn", [DH, NL], BF16, isOutput=False)
    cw_e = nc.declare_dram_parameter("cw", [128, CT * 3], F32, isOutput=False)
    cb_e = nc.declare_dram_parameter("cb", [128, CT], F32, isOutput=False)
    lsel_e = nc.declare_dram_parameter("lsel", [R, 1], BF16, isOutput=False)
    rsel_e = nc.declare_dram_parameter("rsel", [R, 1], BF16, isOutput=False)
    out_e = nc.declare_dram_parameter("out", [D, NL], F32, isOutput=True)
    kvst_e = nc.declare_dram_parameter("kvst", [KV_TOT], BF16, isOutput=True)
    kvx_e = [nc.declare_dram_parameter(f"kvx{r}", [KV_TOT], BF16, isOutput=True)
             for r in range(R)]
    hfst_e = nc.declare_dram_parameter("hfst", [HALO], BF16, isOutput=True)
    hfx_e = nc.declare_dram_parameter("hfx", [R * HALO], BF16, isOutput=True)

    cc_kv_i = nc.dram_tensor("cc_kv_i", [KV_TOT], BF16)
    cc_kv_o = nc.dram_tensor("cc_kv_o", [R * KV_TOT], BF16, addr_space="Shared")
    cc_hf_i = nc.dram_tensor("cc_hf_i", [HALO], BF16)
    cc_hf_o = nc.dram_tensor("cc_hf_o", [R * HALO], BF16, addr_space="Shared")
    GRP = [list(range(R))]

    with tile.TileContext(nc) as tc:
        with (
            tc.tile_pool(name="per", bufs=1) as per,          # persistent
            tc.tile_pool(name="rot", bufs=2) as rot,          # rotating loads
            tc.tile_pool(name="wstr", bufs=3) as wstr,        # weight stream
            tc.tile_pool(name="wk", bufs=2) as wk,            # small working
            tc.tile_pool(name="psA", bufs=4, space="PSUM") as psA,
            tc.tile_pool(name="psAt", bufs=1, space="PSUM") as psAt,
            tc.tile_pool(name="psB", bufs=1, space="PSUM") as psB,
        ):
            # ---------------- Ph0: loads + constants
            xb = []
            for k in range(NK):
                xf = rot.tile([128, NL], F32, tag="xtf")
                nc.sync.dma_start(xf[:], xt_e[128 * k:128 * (k + 1), :])
                b = per.tile([128, NL], BF16, tag=f"xb{k}")
                nc.vector.tensor_copy(b[:], xf[:])
                xb.append(b)
            cs = per.tile([DH, NL], BF16, tag="cs")
            sn = per.tile([DH, NL], BF16, tag="sn")
            nc.sync.dma_start(cs[:], cs_e[:])
            nc.sync.dma_start(sn[:], sn_e[:])
            cw = per.tile([128, CT * 3], F32, tag="cw")
            cb = per.tile([128, CT], F32, tag="cb")
            nc.sync.dma_start(cw[:], cw_e[:])
            nc.sync.dma_start(cb[:], cb_e[:])
            lsel = per.tile([R, 1], BF16, tag="lsel")
            rsel = per.tile([R, 1], BF16, tag="rsel")
            nc.sync.dma_start(lsel[:], lsel_e[:])
            nc.sync.dma_start(rsel[:], rsel_e[:])
            ones64 = per.tile([DH, 1], BF16, tag="o64")
            nc.vector.memset(ones64[:], 1.0)
            ones1_64 = per.tile([1, DH], F32, tag="o1_64")
            nc.vector.memset(ones1_64[:], 1.0)
            ones1_64b = per.tile([1, DH], BF16, tag="o1_64b")
            nc.vector.memset(ones1_64b[:], 1.0)
            ones1_128 = per.tile([1, 128], F32, tag="o1_128")
            nc.vector.memset(ones1_128[:], 1.0)
            ones128 = per.tile([128, 1], BF16, tag="o128")
            nc.vector.memset(ones128[:], 1.0)

            kvst_K = kvst_e[0:KV_K].rearrange("(h p n) -> h p n", h=H, p=DH)
            kvst_V = kvst_e[KV_K:KV_TOT].rearrange("(r p n) -> r p n", r=4, p=128)

            wqkv_kp = wqkv_e[:].rearrange("(k p) c -> p k c", p=128)

            def rope_norm(h, col0, sqrt_scale, sqrt_bias):
                """qkv matmul for head h + RoPE + rms-norm; returns [64, NL] bf16."""
                wh = wstr.tile([128, NK * DH], BF16, tag="wh")
                nc.sync.dma_start(
                    wh[:].rearrange("p (k c) -> p k c", k=NK),
                    wqkv_kp[:, :, col0:col0 + DH])
                ps = psA.tile([DH, NL], F32, tag="mm")
                for k in range(NK):
                    nc.tensor.matmul(ps[:], wh[:, DH * k:DH * (k + 1)], xb[k][:],
                                     start=(k == 0), stop=(k == NK - 1))
                raw = wk.tile([DH, NL], BF16, tag="raw")
                nc.scalar.activation(raw[:], ps[:], AF.Copy)
                sw = wk.tile([DH, NL], BF16, tag="sw")
                nc.sync.dma_start(sw[0:32, :], raw[32:64, 0:NL])
                nc.sync.dma_start(sw[32:64, :], raw[0:32, 0:NL])
                t1 = wk.tile([DH, NL], BF16, tag="t1")
                nc.vector.tensor_mul(t1[:], raw[:], cs[:])
                t2 = wk.tile([DH, NL], BF16, tag="t2")
                nc.gpsimd.tensor_mul(t2[:], sw[:], sn[:])
                rot_ = wk.tile([DH, NL], BF16, tag="rot")
                nc.vector.tensor_add(rot_[:], t1[:], t2[:])
                sq = wk.tile([DH, NL], BF16, tag="sq")
                nc.gpsimd.tensor_mul(sq[:], rot_[:], rot_[:])
                ss = psB.tile([1, NL], F32, tag="ss")
                nc.tensor.matmul(ss[:], ones64[:], sq[:], start=True, stop=True)
                sd = wk.tile([1, NL], F32, tag="sd")
                nc.scalar.activation(sd[:], ss[:], AF.Sqrt,
                                     scale=sqrt_scale, bias=sqrt_bias)
                rc = wk.tile([1, NL], F32, tag="rc")
                nc.vector.reciprocal(rc[:], sd[:])
                bc = psB.tile([DH, NL], F32, tag="bc")
                nc.tensor.matmul(bc[:], ones1_64[:], rc[:], start=True, stop=True)
                o = wk.tile([DH, NL], BF16, tag="nrm")
                nc.vector.tensor_mul(o[:], rot_[:], bc[:])
                return o

            # ---------------- Ph1: K side -> kvst
            for h in range(H):
                kn = rope_norm(h, D + DH * h, 1.0 / DH, 1e-5)
                nc.sync.dma_start(kvst_K[h], kn[:])

            # ---------------- Ph2: V row-major (with ones cols) -> kvst
            wvh = []
            for half in range(2):
                wv = rot.tile([128, NK * 512], BF16, tag=f"wv{half}", bufs=1)
                nc.sync.dma_start(
                    wv[:].rearrange("p (k c) -> p k c", k=NK),
                    wqkv_kp[:, :, 2 * D + 512 * half:2 * D + 512 * (half + 1)])
                wvh.append(wv)
            for rt in range(4):
                va = rot.tile([128, H * 65], BF16, tag="vaug")
                vv = va[:].rearrange("p (h s) -> p h s", s=65)
                nc.vector.memset(vv[:, :, 64:65], 1.0)
                for half in range(2):
                    ps = psA.tile([128, NL], F32, tag="mm")
                    for k in range(NK):
                        nc.tensor.matmul(ps[:], xb[k][:, 128 * rt:128 * (rt + 1)],
                                         wvh[half][:, 512 * k:512 * (k + 1)],
                                         start=(k == 0), stop=(k == NK - 1))
                    dst = vv[:, 8 * half:8 * (half + 1), 0:64]
                    src = ps[:].rearrange("p (h s) -> p h s", s=64)
                    nc.scalar.activation(dst, src, AF.Copy)
                nc.sync.dma_start(kvst_V[rt], va[:])

            # ---------------- Ph3: collective (K/V all-gather)
            nc.sync.dma_start(cc_kv_i[:], kvst_e[:])
            nc.gpsimd.collective_compute(
                "AllGather", OP.bypass, replica_groups=GRP,
                ins=[cc_kv_i.ap().opt()], outs=[cc_kv_o.ap().opt()])
            for r in range(R):
                nc.sync.dma_start(kvx_e[r][:],
                                  cc_kv_o[KV_TOT * r:KV_TOT * (r + 1)])

            # ---------------- Ph4: Q side (overlaps collective)
            qn = []
            for h in range(H):
                q = rope_norm(h, DH * h, 1.0, DH * 1e-5)
                qh = per.tile([DH, NL], BF16, tag=f"qn{h}")
                nc.vector.tensor_copy(qh[:], q[:])
                qn.append(qh)

            # ---------------- Ph6: attention (head pairs u = (2u, 2u+1))
            mp = []
            for u in range(NK):
                p = per.tile([128, NL], BF16, tag=f"mp{u}")
                mp.append(p)
            attn_ctx = tc.tile_pool(name="attn", bufs=3)
            attn = attn_ctx.__enter__()
            for u in range(NK):
                # V^T for both heads of the pair (feature-major, local)
                wvp = wstr.tile([128, NK * 128], BF16, tag="wvp")
                nc.sync.dma_start(
                    wvp[:].rearrange("p (k c) -> p k c", k=NK),
                    wqkv_kp[:, :, 2 * D + 128 * u:2 * D + 128 * (u + 1)])
                vth = []
                for j in range(2):
                    vps = psA.tile([DH, NL], F32, tag="mm")
                    for k in range(NK):
                        nc.tensor.matmul(
                            vps[:], wvp[:, 128 * k + DH * j:128 * k + DH * (j + 1)],
                            xb[k][:], start=(k == 0), stop=(k == NK - 1))
                    vv = attn.tile([DH, NL], BF16, tag=f"vth{j}")
                    nc.scalar.activation(vv[:], vps[:], AF.Copy)
                    vth.append(vv)
                ats = [psAt.tile([65, NL], F32, tag=f"attr{j}", name=f"at{u}_{j}")
                       for j in range(2)]
                for t in range(4 * R):
                    r, c = t // 4, t % 4
                    if c == 0:
                        ksla = attn.tile([DH, 512], BF16, tag="ksla")
                        nc.sync.dma_start(
                            ksla[:],
                            kvx_e[r][DH * NL * 2 * u:DH * NL * (2 * u + 1)]
                            .rearrange("(p n) -> p n", p=DH))
                        kslb = attn.tile([DH, 512], BF16, tag="kslb")
                        nc.sync.dma_start(
                            kslb[:],
                            kvx_e[r][DH * NL * (2 * u + 1):DH * NL * (2 * u + 2)]
                            .rearrange("(p n) -> p n", p=DH))
                        ksl2 = (ksla, kslb)
                    vsl2 = attn.tile([128, 130], BF16, tag="vsl2")
                    nc.sync.dma_start(
                        vsl2[:],
                        kvx_e[r][KV_K + 128 * 1040 * c:KV_K + 128 * 1040 * (c + 1)]
                        .rearrange("(p s) -> p s", p=128)[:, 130 * u:130 * (u + 1)])
                    for j in range(2):
                        h = 2 * u + j
                        sps = psA.tile([128, NL], F32, tag="mm")
                        nc.tensor.matmul(sps[:], ksl2[j][:, 128 * c:128 * (c + 1)],
                                         qn[h][:], start=True, stop=True)
                        e2 = attn.tile([128, NL], BF16, tag="e2")
                        nc.scalar.activation(e2[:], sps[:], AF.Exp, scale=2.0)
                        t1 = attn.tile([128, NL], BF16, tag="wt1")
                        if (2 * t + j) % 3 < 2:
                            nc.vector.tensor_scalar(t1[:], sps[:], 0.0, 1.0,
                                                    OP.max, OP.add)
                        else:
                            rr = attn.tile([128, NL], BF16, tag="rr")
                            nc.scalar.activation(rr[:], sps[:], AF.Relu)
                            nc.vector.tensor_scalar(t1[:], rr[:], 1.0, None, OP.add)
                        t2 = attn.tile([128, NL], BF16, tag="wt2")
                        nc.gpsimd.tensor_mul(t2[:], t1[:], t1[:])
                        w = attn.tile([128, NL], BF16, tag="w")
                        nc.vector.tensor_tensor(w[:], t2[:], e2[:], OP.min)
                        nc.tensor.matmul(ats[j][:], vsl2[:, 65 * j:65 * (j + 1)],
                                         w[:], start=(t == 0), stop=(t == 4 * R - 1))
                for j in range(2):
                    h = 2 * u + j
                    at = ats[j]
                    rs = attn.tile([65, NL], BF16, tag="rs")
                    nc.scalar.activation(rs[64:65, :], at[64:65, :], AF.Copy)
                    r0 = attn.tile([1, NL], BF16, tag="r0")
                    nc.sync.dma_start(r0[:], rs[64:65, 0:NL])
                    ra = attn.tile([1, NL], BF16, tag="ra")
                    nc.vector.tensor_scalar(ra[:], r0[:], 1e-6, None, OP.add)
                    rc = attn.tile([1, NL], BF16, tag="rcp")
                    with nc.allow_low_precision(reason="attn rowsum recip in bf16"):
                        nc.vector.reciprocal(rc[:], ra[:])
                    bc = psB.tile([DH, NL], F32, tag="bc")
                    nc.tensor.matmul(bc[:], ones1_64b[:], rc[:], start=True, stop=True)
                    bcs = attn.tile([DH, NL], BF16, tag="bcs")
                    nc.scalar.activation(bcs[:], bc[:], AF.Copy)
                    cc = attn.tile([DH, NL], BF16, tag="cs_att")
                    nc.vector.tensor_mul(cc[:], at[0:DH, :], bcs[:])
                    if j == 0:
                        nc.gpsimd.tensor_sub(mp[u][0:DH, :], cc[:], vth[j][:])
                    else:
                        m = attn.tile([DH, NL], BF16, tag="modd")
                        nc.gpsimd.tensor_sub(m[:], cc[:], vth[j][:])
                        nc.sync.dma_start(mp[u][DH:128, :], m[:, 0:NL])

            attn_ctx.__exit__(None, None, None)
            # ---------------- Ph8: W_o + residual + rms -> QI
            zt = []
            ss2 = psB.tile([1, NL], F32, tag="ss")
            wo_kp = wo_e[:].rearrange("(k p) c -> p k c", p=128)
            for mi in range(NK):
                wom = wstr.tile([128, NK * 128], BF16, tag="wom")
                nc.sync.dma_start(
                    wom[:].rearrange("p (k c) -> p k c", k=NK),
                    wo_kp[:, :, 128 * mi:128 * (mi + 1)])
                ps = psA.tile([128, NL], F32, tag="mm")
                for k in range(NK):
                    nc.tensor.matmul(ps[:], wom[:, 128 * k:128 * (k + 1)], mp[k][:],
                                     start=(k == 0), stop=(k == NK - 1))
                z = per.tile([128, NL], BF16, tag=f"zf{mi}")
                nc.vector.tensor_add(z[:], ps[:], xb[mi][:])
                zt.append(z)
                sq = wk.tile([128, NL], BF16, tag="sq2")
                nc.vector.tensor_mul(sq[:], z[:], z[:])
                nc.tensor.matmul(ss2[:], ones128[:], sq[:],
                                 start=(mi == 0), stop=(mi == NK - 1))
            sd2 = wk.tile([1, NL], F32, tag="sd")
            nc.scalar.activation(sd2[:], ss2[:], AF.Sqrt, scale=1.0 / D, bias=1e-5)
            rc2 = wk.tile([1, NL], F32, tag="rc")
            nc.vector.reciprocal(rc2[:], sd2[:])
            bc2 = psB.tile([128, NL], F32, tag="bc")
            nc.tensor.matmul(bc2[:], ones1_128[:], rc2[:], start=True, stop=True)
            qib = []
            for mi in range(NK):
                qb = per.tile([128, NL], BF16, tag=f"qib{mi}")
                nc.vector.tensor_mul(qb[:], zt[mi][:], bc2[:])
                qib.append(qb)

            # ---------------- Ph9: SwiGLU FFN up + Hf
            ffn_ctx = tc.tile_pool(name="ffn", bufs=1)
            ffn = ffn_ctx.__enter__()
            wup_kp = wup_e[:].rearrange("(k p) c -> p k c", p=128)
            hf = []
            for c in range(CT):
                wgm = wstr.tile([128, NK * 128], BF16, tag="wgm")
                nc.sync.dma_start(
                    wgm[:].rearrange("p (k c) -> p k c", k=NK),
                    wup_kp[:, :, 128 * c:128 * (c + 1)])
                gps = psA.tile([128, NL], F32, tag="mm")
                for k in range(NK):
                    nc.tensor.matmul(gps[:], wgm[:, 128 * k:128 * (k + 1)], qib[k][:],
                                     start=(k == 0), stop=(k == NK - 1))
                gs = wk.tile([128, NL], BF16, tag="gs")
                nc.scalar.activation(gs[:], gps[:], AF.Silu)
                wum = wstr.tile([128, NK * 128], BF16, tag="wum")
                nc.sync.dma_start(
                    wum[:].rearrange("p (k c) -> p k c", k=NK),
                    wup_kp[:, :, INNER + 128 * c:INNER + 128 * (c + 1)])
                ups = psA.tile([128, NL], F32, tag="mm")
                for k in range(NK):
                    nc.tensor.matmul(ups[:], wum[:, 128 * k:128 * (k + 1)], qib[k][:],
                                     start=(k == 0), stop=(k == NK - 1))
                f = ffn.tile([128, NL + 2], BF16, tag=f"hf{c}")
                nc.vector.tensor_mul(f[:, 1:NL + 1], gs[:], ups[:])
                hf.append(f)
                # stage boundary cols for the halo exchange
                nc.sync.dma_start(hfst_e[256 * c:256 * c + 128], f[:, 1:2])
                nc.sync.dma_start(hfst_e[256 * c + 128:256 * c + 256], f[:, NL:NL + 1])

            # ---------------- Ph10: halo collective + rank-select
            nc.sync.dma_start(cc_hf_i[:], hfst_e[:])
            nc.gpsimd.collective_compute(
                "AllGather", OP.bypass, replica_groups=GRP,
                ins=[cc_hf_i.ap().opt()], outs=[cc_hf_o.ap().opt()])
            nc.sync.dma_start(hfx_e[:], cc_hf_o[:])
            hfx_R = hfx_e[:].rearrange("(r e) -> r e", r=R)
            for c in range(CT):
                hsbc = wk.tile([R, 256], BF16, tag="hsbc")
                nc.sync.dma_start(hsbc[:], hfx_R[:, 256 * c:256 * (c + 1)])
                # left halo = lsel . lastcols ; right halo = rsel . firstcols
                hpsL = psB.tile([1, 128], F32, tag="bc")
                nc.tensor.matmul(hpsL[:], lsel[:], hsbc[:, 128:256],
                                 start=True, stop=True)
                hpsR = psB.tile([1, 128], F32, tag="bc")
                nc.tensor.matmul(hpsR[:], rsel[:], hsbc[:, 0:128],
                                 start=True, stop=True)
                hrowL = wk.tile([1, 128], BF16, tag="hrowL")
                nc.scalar.activation(hrowL[:], hpsL[:], AF.Copy)
                hrowR = wk.tile([1, 128], BF16, tag="hrowR")
                nc.scalar.activation(hrowR[:], hpsR[:], AF.Copy)
                nc.sync.dma_start(hf[c][:, 0:1], hrowL[0:1, 0:128])
                nc.sync.dma_start(hf[c][:, NL + 1:NL + 2], hrowR[0:1, 0:128])

            # ---------------- Ph11: depthwise conv + silu -> hc
            hc = []
            for c in range(CT):
                a = wk.tile([128, NL], BF16, tag="cva")
                nc.gpsimd.tensor_scalar_mul(a[:], hf[c][:, 0:NL], cw[:, 3 * c:3 * c + 1])
                b = wk.tile([128, NL], BF16, tag="cvb")
                nc.vector.scalar_tensor_tensor(
                    b[:], hf[c][:, 1:NL + 1], cw[:, 3 * c + 1:3 * c + 2], a[:],
                    OP.mult, OP.add)
                d2 = wk.tile([128, NL], BF16, tag="cvd2")
                nc.gpsimd.tensor_scalar_mul(d2[:], hf[c][:, 2:NL + 2],
                                            cw[:, 3 * c + 2:3 * c + 3])
                d = wk.tile([128, NL], BF16, tag="cvd")
                nc.gpsimd.tensor_add(d[:], d2[:], b[:])
                nc.scalar.activation(hf[c][:, 1:NL + 1], d[:], AF.Silu,
                                     bias=cb[:, c:c + 1])
                hc.append(hf[c])

            # ---------------- Ph12: W_down + residual + final rms -> out
            ft = []
            ss3 = psB.tile([1, NL], F32, tag="ss")
            wdn_kp = wdn_e[:].rearrange("(k p) c -> p k c", p=128)
            for mi in range(NK):
                wdm = wstr.tile([128, CT * 128], BF16, tag="wdm", bufs=2)
                nc.sync.dma_start(
                    wdm[:].rearrange("p (k c) -> p k c", k=CT),
                    wdn_kp[:, :, 128 * mi:128 * (mi + 1)])
                ps = psA.tile([128, NL], F32, tag="mm")
                for c in range(CT):
                    nc.tensor.matmul(ps[:], wdm[:, 128 * c:128 * (c + 1)],
                                     hc[c][:, 1:NL + 1],
                                     start=(c == 0), stop=(c == CT - 1))
                fz = per.tile([128, NL], BF16, tag=f"zf{mi}")
                nc.vector.tensor_add(fz[:], ps[:], qib[mi][:])
                ft.append(fz)
                sq = wk.tile([128, NL], BF16, tag="sq2")
                nc.vector.tensor_mul(sq[:], fz[:], fz[:])
                nc.tensor.matmul(ss3[:], ones128[:], sq[:],
                                 start=(mi == 0), stop=(mi == NK - 1))
            sd3 = wk.tile([1, NL], F32, tag="sd")
            nc.scalar.activation(sd3[:], ss3[:], AF.Sqrt, scale=1.0 / D, bias=1e-5)
            rc3 = wk.tile([1, NL], F32, tag="rc")
            nc.vector.reciprocal(rc3[:], sd3[:])
            bc3 = psB.tile([128, NL], F32, tag="bc")
            nc.tensor.matmul(bc3[:], ones1_128[:], rc3[:], start=True, stop=True)
            for mi in range(NK):
                o = wk.tile([128, NL], F32, tag="ofin")
                nc.vector.tensor_mul(o[:], ft[mi][:], bc3[:])
                nc.sync.dma_start(out_e[128 * mi:128 * (mi + 1), :], o[:])
            ffn_ctx.__exit__(None, None, None)

    fix_sync_waits(nc)
    return nc


_NC = None


def kernel(Q_in, cos, sin, W_qkv, W_o, W_up, conv_w, conv_b, W_down):
    global _NC
    if _NC is None:
        _NC = build_kernel()
    nc = _NC

    Q_in = np.asarray(Q_in, dtype=np.float32)
    cos = np.asarray(cos, dtype=np.float32)
    sin = np.asarray(sin, dtype=np.float32)
    wqkv = np.ascontiguousarray(np.asarray(W_qkv, np.float32).astype(bfdt))
    wo = np.ascontiguousarray(np.asarray(W_o, np.float32).astype(bfdt))
    wup = np.ascontiguousarray(np.asarray(W_up, np.float32).astype(bfdt))
    wdn = np.ascontiguousarray(np.asarray(W_down, np.float32).astype(bfdt))
    cwt = np.asarray(conv_w, np.float32)[:, 0, :].T  # [2816, 3]
    cw = np.ascontiguousarray(
        cwt.reshape(CT, 128, 3).transpose(1, 0, 2).reshape(128, CT * 3))
    cb = np.ascontiguousarray(np.asarray(conv_b, np.float32).reshape(CT, 128).T)

    in_maps = []
    for c in range(R):
        rows = slice(NL * c, NL * (c + 1))
        xt = np.ascontiguousarray(Q_in[0, rows, :].T)
        csA = cos[rows, 0:32].T.astype(bfdt)      # [32, NL]
        snA = sin[rows, 0:32].T.astype(bfdt)
        csf = np.ascontiguousarray(np.concatenate([csA, csA], axis=0))
        snf = np.ascontiguousarray(np.concatenate([-snA, snA], axis=0))
        ls = np.zeros((R, 1), bfdt)
        rs = np.zeros((R, 1), bfdt)
        if c > 0:
            ls[c - 1, 0] = 1.0
        if c < R - 1:
            rs[c + 1, 0] = 1.0
        in_maps.append({
            "xt": xt, "wqkv": wqkv, "wo": wo, "wup": wup, "wdn": wdn,
            "cs": csf, "sn": snf, "cw": cw, "cb": cb, "lsel": ls, "rsel": rs,
        })

    import os
    trace = bool(os.environ.get("KTRACE"))
    res = run_bass_kernel_spmd(nc, in_maps, core_ids=list(range(R)), trace=trace)
    if trace:
        print(f"HW exec time: {res.exec_time_ns} ns")
    out = np.empty((1, N, D), np.float32)
    for c in range(R):
        out[0, NL * c:NL * (c + 1), :] = np.asarray(res.results[c]["out"]).T
    return out

